# revision 1
# baseline (speedup 1.0000x reference)
"""Trainium2 Bass kernel for nn_CapsuleLayer (dynamic routing).

Problem:  u_hat = einsum('bri,crio->cbro', x, W);  3 routing iterations
          (softmax over R, weighted sum, squash, agreement update).
Shapes:   x [256, 1152, 8] f32, W [10, 1152, 8, 16] f32 ->
          out [10, 256, 1, 1, 16] f32.

Strategy (8 NeuronCores, data-parallel over batch, B_loc = 32/core):
  * never materialize u_hat (189 MB) in HBM;
  * s-sums   : PE matmuls, K = 128-row r-blocks, i via 8 accumulating
               matmuls, y = softmax-weights * x built on DVE/GPSIMD;
  * agreement: PE matmuls with block-diagonal v stationaries streaming a
               (c,o)-partition W copy (streamed from HBM per iteration),
               then fused multiply + i-tree, L accumulated in fp32;
  * softmax  : ACT exp with accumulated Z, weights transposed to r-block
               partitions with PE transposes.
All arithmetic fp32 (bf16 anywhere in the logit path measurably breaks
the output tolerance: ~1e-2 rel err per bf16-rounded component).
"""

import sys
from contextlib import ExitStack

import numpy as np

sys.path.insert(0, "/opt/trn_rl_repo")

import concourse.bacc as bacc
import concourse.bass as bass
import concourse.mybir as mybir
import concourse.tile as tile
from concourse.bass_utils import run_bass_kernel_spmd

F32 = mybir.dt.float32
F16 = mybir.dt.float16
MUL = mybir.AluOpType.mult
ADD = mybir.AluOpType.add

B, R, I, C, O = 256, 1152, 8, 10, 16
NC = 8
BL = B // NC          # 32 batch per core
Q = R // 128          # 9 r-blocks of 128
CO = C * O            # 160
RI = R * I            # 9216
EPS = 1e-7
GCH = 1024            # g-matmul free-dim chunk (elements of (r,i))
NG = RI // GCH        # 9 chunks
W3 = 3                # (c,b) waves


def build_nc(debug=False):
    nc = bacc.Bacc("TRN2", target_bir_lowering=False, debug=debug)

    xtr_d = nc.declare_dram_parameter("xtr", [128, Q, I, BL], F32, isOutput=False)
    wfr_d = nc.declare_dram_parameter("wfr", [128, Q, I, CO], F32, isOutput=False)
    wt_d = nc.declare_dram_parameter("wt", [4, 16, 3, RI], F16, isOutput=False)
    xrep_d = nc.declare_dram_parameter("xrep", [128, RI], F16, isOutput=False)
    ident_d = nc.declare_dram_parameter("ident", [128, 128], F32, isOutput=False)
    out_d = nc.declare_dram_parameter("out", [C, O, BL], F32, isOutput=True)

    with tile.TileContext(nc) as tc, ExitStack() as ctx:
        res = ctx.enter_context(tc.tile_pool(name="res", bufs=1))
        cwp = ctx.enter_context(tc.tile_pool(name="cwp", bufs=2))
        yp = ctx.enter_context(tc.tile_pool(name="yp", bufs=2))
        wtp = ctx.enter_context(tc.tile_pool(name="wtp", bufs=2))
        gmp = ctx.enter_context(tc.tile_pool(name="gmp", bufs=2))
        trp = ctx.enter_context(tc.tile_pool(name="trp", bufs=2))
        smp = ctx.enter_context(tc.tile_pool(name="smp", bufs=1))
        psS = ctx.enter_context(
            tc.tile_pool(name="psS", bufs=1, space=bass.MemorySpace.PSUM)
        )
        psG = ctx.enter_context(
            tc.tile_pool(name="psG", bufs=2, space=bass.MemorySpace.PSUM)
        )
        psT = ctx.enter_context(
            tc.tile_pool(name="psT", bufs=2, space=bass.MemorySpace.PSUM)
        )
        psN = ctx.enter_context(
            tc.tile_pool(name="psN", bufs=1, space=bass.MemorySpace.PSUM)
        )

        # ---- resident tensors -------------------------------------
        xtr = res.tile([128, Q, I, BL], F32)
        wfr = res.tile([128, Q, I, CO], F32)
        xrep = res.tile([128, RI], F16)
        ident = res.tile([128, 128], F32)
        L = res.tile([128, W3, R], F32)
        cwT = res.tile([128, Q, W3, 128], F32)
        Z = res.tile([128, W3], F32)
        Zi = res.tile([128, W3], F32)
        vblk = res.tile([128, 3, 32], F16)   # v[c, b, o] staged at rows 32P+o
        ones16 = res.tile([16, 1], F32)
        v_sb = res.tile([16, C, BL], F32)    # squash output, [o, c, b]

        nc.sync.dma_start(xtr[:], xtr_d[:])
        nc.sync.dma_start(wfr[:], wfr_d[:])
        nc.sync.dma_start(xrep[:], xrep_d[:])
        nc.sync.dma_start(ident[:], ident_d[:])
        nc.vector.memset(L[:], 0.0)
        nc.vector.memset(ones16[:], 1.0)

        # ---------------------------------------------------------------
        def build_y(c):
            """y_c[rr, q, i, b] = cw[c, b, 128q+rr] * x[b, 128q+rr, i].

            cw comes from cwT (r-block partitions); (c,b) column index in
            cwT is p = 32k + b, where class c = 4w + k (w=2: c = 8 + k).
            """
            w = min(c // 4, 2)
            k = c - 4 * w
            y_c = yp.tile([128, Q, I, BL], F32, tag="y")
            cw_src = (
                cwT[:, :, w, 32 * k : 32 * k + 32]
                .unsqueeze(2)
                .broadcast_to([128, Q, I, BL])
            )
            eng = nc.vector if c % 2 == 0 else nc.gpsimd
            eng.tensor_tensor(y_c, xtr[:], cw_src, MUL)
            return y_c

        def s_pass(it):
            """sps[o, c, b] = sum_{r,i} rhs_c[r, i, b] * W[c, r, i, o]."""
            sps = psS.tile([16, C, BL], F32, tag="sps")
            for c in range(C):
                y_c = None if it == 0 else build_y(c)
                for q in range(Q):
                    for i in range(I):
                        rhs = xtr[:, q, i, :] if it == 0 else y_c[:, q, i, :]
                        nc.tensor.matmul(
                            sps[:, c, :],
                            wfr[:, q, i, c * O : (c + 1) * O],
                            rhs,
                            start=(q == 0 and i == 0),
                            stop=(q == Q - 1 and i == I - 1),
                        )
            return sps

        # ---------------------------------------------------------------
        def squash(sps, it):
            """v_sb = squash(s) over o;  it==0 folds the uniform 1/R weight."""
            sq = smp.tile([16, C * BL], F32, tag="sq")
            nc.scalar.activation(
                sq[:],
                sps[:].rearrange("o c b -> o (c b)"),
                mybir.ActivationFunctionType.Square,
            )
            snps = psN.tile([1, C * BL], F32, tag="snps")
            nc.tensor.matmul(snps[:], ones16[:], sq[:], start=True, stop=True)
            sn = smp.tile([1, C * BL], F32, tag="sn")
            if it == 0:
                nc.vector.tensor_scalar_mul(sn[:], snps[:], 1.0 / (R * R))
            else:
                nc.vector.tensor_copy(sn[:], snps[:])
            u1 = smp.tile([1, C * BL], F32, tag="u1")
            u2 = smp.tile([1, C * BL], F32, tag="u2")
            u3 = smp.tile([1, C * BL], F32, tag="u3")
            u4 = smp.tile([1, C * BL], F32, tag="u4")
            f = smp.tile([1, C * BL], F32, tag="f")
            nc.vector.tensor_scalar_add(u1[:], sn[:], EPS)
            nc.scalar.activation(u2[:], u1[:], mybir.ActivationFunctionType.Sqrt)
            nc.vector.tensor_scalar_add(u3[:], sn[:], 1.0)
            nc.vector.tensor_tensor(u4[:], u2[:], u3[:], MUL)
            nc.vector.reciprocal(u1[:], u4[:])
            nc.vector.tensor_tensor(f[:], sn[:], u1[:], MUL)
            if it == 0:
                nc.vector.tensor_scalar_mul(f[:], f[:], 1.0 / R)
            frep = smp.tile([16, C * BL], F32, tag="frep")
            nc.gpsimd.partition_broadcast(frep[:], f[:])
            nc.vector.tensor_tensor(
                v_sb[:].rearrange("o c b -> o (c b)"),
                sps[:].rearrange("o c b -> o (c b)"),
                frep[:],
                MUL,
            )

        def fill_vblk():
            """Stage v as g-matmul stationaries (fp16): slot cc of strip P
            holds class 2P+cc; slot 2 of strip P holds class 8+P (P<2)."""
            for c in range(C):
                P, slot = (c // 2, c % 2) if c < 8 else (c - 8, 2)
                nc.vector.tensor_copy(
                    vblk[32 * P : 32 * P + 16, slot, :], v_sb[:, c, :]
                )

        # ---------------------------------------------------------------
        def agreement():
            """L[p, w, r] += sum_o v[c,b,o]*W[c,r,i,o] (*) x[b,r,i], sum_i."""
            for n0 in range(NG):
                off = n0 * GCH
                wt_t = wtp.tile([128, 3, GCH], F16, tag="wt")
                for P in range(4):
                    nc.sync.dma_start(
                        wt_t[32 * P : 32 * P + 16, :, :],
                        wt_d[P, :, :, off : off + GCH],
                    )
                for w in range(W3):
                    gps = psG.tile([128, GCH], F32, tag="gps")
                    npart = 128 if w < 2 else 64
                    for k in range(4 if w < 2 else 2):
                        c = 4 * w + k
                        P, slot = (c // 2, c % 2) if c < 8 else (c - 8, 2)
                        base = 32 * P
                        for sub in range(0, GCH, 512):
                            nc.tensor.matmul(
                                gps[32 * k : 32 * k + 32, sub : sub + 512],
                                vblk[base : base + 16, slot, :],
                                wt_t[base : base + 16, slot, sub : sub + 512],
                                start=True,
                                stop=True,
                                tile_position=(base, 32 * k),
                            )
                    # fused drain-multiply:  gm = gps * xrep   (fp32, DVE)
                    gm = gmp.tile([128, GCH], F32, tag="gm")
                    nc.vector.tensor_tensor(
                        gm[:npart, :],
                        gps[:npart, :],
                        xrep[:npart, off : off + GCH],
                        MUL,
                    )
                    # i-reduction tree (8 -> 4 -> 2 -> 1) on GPSIMD
                    nr = GCH // I
                    l1 = trp.tile([128, GCH // 2], F32, tag="l1")
                    l2 = trp.tile([128, GCH // 4], F32, tag="l2")
                    a = trp.tile([128, GCH // 8], F32, tag="a")
                    gmv = gm.rearrange("p (r i) -> p r i", i=I)
                    l1v = l1.rearrange("p (r i) -> p r i", i=4)
                    l2v = l2.rearrange("p (r i) -> p r i", i=2)
                    nc.gpsimd.tensor_tensor(
                        l1v[:npart], gmv[:npart, :, 0:4], gmv[:npart, :, 4:8], ADD
                    )
                    nc.gpsimd.tensor_tensor(
                        l2v[:npart], l1v[:npart, :, 0:2], l1v[:npart, :, 2:4], ADD
                    )
                    nc.gpsimd.tensor_tensor(
                        a[:npart], l2v[:npart, :, 0], l2v[:npart, :, 1], ADD
                    )
                    r0 = off // I
                    nc.vector.tensor_tensor(
                        L[:npart, w, r0 : r0 + nr],
                        L[:npart, w, r0 : r0 + nr],
                        a[:npart],
                        ADD,
                    )

        # ---------------------------------------------------------------
        def softmax_transpose():
            """cw = softmax_r(L) per (c,b); write transposed into cwT."""
            for w in range(W3):
                cwv = cwp.tile([128, R], F32, tag="cw")
                nc.scalar.activation(
                    cwv[:],
                    L[:, w, :],
                    mybir.ActivationFunctionType.Exp,
                    accum_out=Z[:, w : w + 1],
                )
                nc.vector.reciprocal(Zi[:, w : w + 1], Z[:, w : w + 1])
                nc.vector.tensor_scalar_mul(cwv[:], cwv[:], Zi[:, w : w + 1])
                for q in range(Q):
                    tps = psT.tile([128, 128], F32, tag="tps")
                    nc.tensor.transpose(
                        tps[:], cwv[:, 128 * q : 128 * (q + 1)], ident[:]
                    )
                    nc.scalar.copy(cwT[:, q, w, :], tps[:])

        # =========================== flow ==============================
        for it in range(3):
            if it > 0:
                softmax_transpose()
            sps = s_pass(it)
            squash(sps, it)
            if it < 2:
                fill_vblk()
                agreement()

        nc.sync.dma_start(out_d[:].rearrange("c o b -> o c b"), v_sb[:])

    nc.compile()
    return nc


# =================== host-side prep / entry point =====================

def _prep_shared(W):
    """Per-problem constant tensors (replicated on every core)."""
    W = np.ascontiguousarray(W, np.float32)
    # wfr[rr, q, i, 16c+o] = W[c, 128q+rr, i, o]
    wfr = np.ascontiguousarray(
        W.reshape(C, Q, 128, I, O).transpose(2, 1, 3, 0, 4).reshape(128, Q, I, CO)
    )
    # wt[P, o, slot, 8r+i]: slot cc<2 -> W[2P+cc]; slot 2 -> W[8+P] (P<2).
    wt = np.zeros((4, 16, 3, RI), np.float16)
    for P in range(4):
        for cc in range(2):
            wt[P, :, cc, :] = W[2 * P + cc].transpose(2, 0, 1).reshape(O, RI)
    for P in range(2):
        wt[P, :, 2, :] = W[8 + P].transpose(2, 0, 1).reshape(O, RI)
    ident = np.eye(128, dtype=np.float32)
    return wfr, wt, ident


def _prep_core(x_shard):
    """Per-core tensors for one 32-batch shard: xtr and xrep."""
    xs = np.ascontiguousarray(x_shard, np.float32)       # [32, 1152, 8]
    xtr = np.ascontiguousarray(
        xs.reshape(BL, Q, 128, I).transpose(2, 1, 3, 0)
    )                                                     # [128, Q, I, 32]
    flat = xs.reshape(BL, RI)                             # [b, 8r+i]
    xrep = np.ascontiguousarray(
        flat[np.arange(128) % BL].astype(np.float16)
    )                                                     # [128, RI]
    return xtr, xrep


_NC_CACHE = {}


def kernel(x, W):
    x = np.asarray(x, np.float32)
    W = np.asarray(W, np.float32)
    if "nc" not in _NC_CACHE:
        _NC_CACHE["nc"] = build_nc()
    nc = _NC_CACHE["nc"]

    wfr, wt, ident = _prep_shared(W)
    in_maps = []
    for m in range(NC):
        xtr, xrep = _prep_core(x[m * BL : (m + 1) * BL])
        in_maps.append(
            {"xtr": xtr, "wfr": wfr, "wt": wt, "xrep": xrep, "ident": ident}
        )

    res = run_bass_kernel_spmd(nc, in_maps, list(range(NC)))
    out = np.empty((C, B, 1, 1, O), np.float32)
    for m in range(NC):
        o = res.results[m]["out"]                         # [C, O, BL]
        out[:, m * BL : (m + 1) * BL, 0, 0, :] = np.asarray(o).transpose(0, 2, 1)
    return out


if __name__ == "__main__":
    d = np.load("/root/problem/ref_data.npz")
    got = kernel(d["x"], d["W"])
    exp = d["expected"]
    err = np.abs(got - exp).max() / np.abs(exp).max()
    print("Relative error:", err)



# revision 18
# speedup vs baseline: 1.1888x; 1.1888x over previous
"""Trainium2 Bass kernel for nn_CapsuleLayer (dynamic routing), v3.

Problem:  u_hat = einsum('bri,crio->cbro', x, W);  3 routing iterations
          (softmax over R, weighted sum, squash, agreement update).
Shapes:   x [256, 1152, 8] f32, W [10, 1152, 8, 16] f32 ->
          out [10, 256, 1, 1, 16] f32.

Strategy (8 NeuronCores, data-parallel over batch, B_loc = 32/core):
  * all matmul operands fp16 (PE 16-bit streams 1 col/cycle vs fp32 4x);
    accumulation fp32 in PSUM, logits L accumulate fp32 in SBUF;
  * s-pass it0: x-chunk stationaries [128,32] against W moving [128,160]
    giving s[b, co] directly; it1/2: shared W stationaries [128,128]
    covering classes 0-7 with a [128,256] moving of per-class weighted x
    (y8); the (class,class) diagonal blocks of the PSUM result are s;
  * every engine op needs a 32-aligned partition base, so per-class
    16-row data lives in padded 32-row slots (top 16 = data, bottom 16 =
    zeros); the s diagonal is rearranged into that form with PE
    permutation matmuls, per-class norms go through one concatenated
    [128,96] square tile and a single [10,96]-output matmul;
  * agreement: per-wave padded block-diagonal v stationary (3 LDWEIGHTS
    per agreement) streaming resident padded wt in 1024-col chunks;
    U*x on DVE, i-reduction tree on GpSimd in fp16, L accumulated fp32;
  * softmax: ACT exp (fp32, accumulated Z); 1/Z is folded into the PE
    transpose by using diag(1/Z) as the matmul rhs; cwT stored fp16.
"""

import sys
from contextlib import ExitStack

import numpy as np

sys.path.insert(0, "/opt/trn_rl_repo")

import concourse.bacc as bacc
import concourse.bass as bass
import concourse.mybir as mybir
import concourse.tile as tile
from concourse.bass_utils import run_bass_kernel_spmd

F32 = mybir.dt.float32
F16 = mybir.dt.float16
MUL = mybir.AluOpType.mult
ADD = mybir.AluOpType.add
AF = mybir.ActivationFunctionType

B, R, I, C, O = 256, 1152, 8, 10, 16
NC = 8
BL = B // NC          # 32 batch per core
Q = R // 128          # 9 r-blocks of 128
RI = R * I            # 9216
GCH = 1024            # agreement (r,i) chunk
NG = RI // GCH        # 9 chunks
EPS = 1e-7
W_ROWS = (128, 128, 64)   # U-matmul output rows per wave ((k,b) pairs)


def build_nc(debug=False):
    nc = bacc.Bacc("TRN2", target_bir_lowering=False, debug=debug)

    def din(name, shape, dt=F16):
        return nc.declare_dram_parameter(name, shape, dt, isOutput=False)

    xtr_d = din("xtr", [128, Q, I, BL])
    wfr8_d = din("wfr8", [128, Q, I, 128])
    wfr2_d = din("wfr2", [128, Q, I, 32])
    xrep_d = din("xrep", [128, RI])
    wtg0_d = din("wtg0", [128, RI])
    wtg1_d = din("wtg1", [128, RI])
    wtg2_d = din("wtg2", [64, RI])
    id16_d = din("id16", [128, 128])
    id32_d = din("id32", [128, 128], F32)
    p0_d = din("p0", [128, 128], F32)
    p1_d = din("p1", [128, 128], F32)
    p2_d = din("p2", [32, 64], F32)
    e10_d = din("e10", [128, C], F32)
    efa_d = din("efa", [C, 128], F32)
    efb_d = din("efb", [C, 128], F32)
    efc_d = din("efc", [C, 64], F32)
    out_d = nc.declare_dram_parameter("out", [C, O, BL], F32, isOutput=True)

    with tile.TileContext(nc) as tc, ExitStack() as ctx:
        res = ctx.enter_context(tc.tile_pool(name="res", bufs=1))
        cwp = ctx.enter_context(tc.tile_pool(name="cwp", bufs=3))
        gmp = ctx.enter_context(tc.tile_pool(name="gmp", bufs=2))
        trp = ctx.enter_context(tc.tile_pool(name="trp", bufs=2))
        smp = ctx.enter_context(tc.tile_pool(name="smp", bufs=1))
        psU = ctx.enter_context(
            tc.tile_pool(name="psU", bufs=2, space=bass.MemorySpace.PSUM)
        )
        psP = ctx.enter_context(
            tc.tile_pool(name="psP", bufs=1, space=bass.MemorySpace.PSUM)
        )

        # ---- resident tensors -------------------------------------
        xtr = res.tile([128, Q, I, BL], F16)
        wfr8 = res.tile([128, Q, I, 128], F16)
        wfr2 = res.tile([128, Q, I, 32], F16)
        xrep = res.tile([128, RI], F16)
        wtg0 = res.tile([128, RI], F16)
        wtg1 = res.tile([128, RI], F16)
        wtg2 = res.tile([64, RI], F16)
        id16 = res.tile([128, 128], F16)
        id32 = res.tile([128, 128], F32)
        p0 = res.tile([128, 128], F32)
        p1 = res.tile([128, 128], F32)
        p2 = res.tile([32, 64], F32)
        e10 = res.tile([128, C], F32)
        efa = res.tile([C, 128], F32)
        efb = res.tile([C, 128], F32)
        efc = res.tile([C, 64], F32)
        L = res.tile([128, 3, R], F32)
        cwT = res.tile([128, Q, 3, 128], F16)
        y8 = res.tile([128, Q, I, 256], F16)
        y2 = res.tile([128, Q, I, 64], F16)
        # padded per-class storage: rows 32k..32k+16 = data, rest zero
        VAB0 = res.tile([128, 128], F16)  # v stationary, classes 0-3
        VAB1 = res.tile([128, 128], F16)  # classes 4-7
        VC = res.tile([64, 64], F16)      # classes 8-9
        s1p0 = res.tile([128, BL], F32)   # padded s, classes 0-3
        s1p1 = res.tile([128, BL], F32)   # classes 4-7
        s2p = res.tile([64, BL], F32)     # classes 8-9
        sqcat = res.tile([128, 96], F32)  # squares, 3 col-blocks
        v0p = res.tile([BL, 256], F16)    # it0 v, col-padded (c, 32)
        v0p2 = res.tile([BL, 64], F16)

        # PSUM (8 banks x 2KB). psU: 2 bufs x [128,1024]f32 = banks 0-3.
        # ps1 bank0 (cols 0:512): psA s-accumulator. bank1 (cols 512:1024):
        # psB/psC accumulators + single-shot outputs. A group's start=True
        # clears has_written bank-wide, so concurrently-accumulating groups
        # (psA vs psB) sit in different banks; single-shot outputs only
        # lose has_written bits, never data.
        ps1 = psP.tile([128, 1024], F32)
        psA = ps1[:, 0:256]
        psB = ps1[0:32, 512:576]
        # psC's two concurrently-accumulating groups must sit in different
        # banks (start=True clears has_written bank-wide)
        psC0 = ps1[0:BL, 256:384]        # bank 0 (idle during it0)
        psC1 = ps1[0:BL, 576:608]        # bank 1
        psP2 = ps1[0:64, 736:800]        # permuted sB
        psn = ps1[0:C, 800:896]          # [10, 96] norms
        psFa = ps1[:, 896:928]           # padded frep / it0 transposed v
        psFb = ps1[:, 928:960]
        psFc = ps1[0:64, 960:992]
        # transpose slots in separate banks: PE writing a bank while another
        # engine reads the same bank is fatal. Also hold permuted sA halves.
        psT6 = psP.tile([128, 256], F32)
        psT7 = psP.tile([128, 256], F32)

        # input DMAs: per-q for the s-pass tensors so compute starts early
        for q in range(Q):
            nc.sync.dma_start(xtr[:, q], xtr_d[:, q])
            nc.sync.dma_start(wfr8[:, q], wfr8_d[:, q])
            nc.sync.dma_start(wfr2[:, q], wfr2_d[:, q])
        for t, d in ((id16, id16_d), (id32, id32_d), (p0, p0_d), (p1, p1_d),
                     (p2, p2_d), (e10, e10_d), (efa, efa_d), (efb, efb_d),
                     (efc, efc_d), (wtg0, wtg0_d), (xrep, xrep_d),
                     (wtg1, wtg1_d), (wtg2, wtg2_d)):
            nc.sync.dma_start(t[:], d[:])
        nc.vector.memset(L[:], 0.0)
        nc.gpsimd.memset(VAB0[:], 0.0)
        nc.gpsimd.memset(VAB1[:], 0.0)
        nc.gpsimd.memset(VC[:], 0.0)
        nc.gpsimd.memset(s1p0[:], 0.0)
        nc.gpsimd.memset(s1p1[:], 0.0)
        nc.gpsimd.memset(s2p[:], 0.0)
        nc.gpsimd.memset(sqcat[:], 0.0)
        nc.gpsimd.memset(v0p[:], 0.0)
        nc.gpsimd.memset(v0p2[:], 0.0)

        # ---------------------------------------------------------------
        def f_chain(snc, p, n):
            """f = (sn/(1+sn)) / sqrt(sn+eps) elementwise on [p, n]."""
            u1 = smp.tile([p, n], F32, tag="u1")
            u2 = smp.tile([p, n], F32, tag="u2")
            u3 = smp.tile([p, n], F32, tag="u3")
            f = smp.tile([p, n], F32, tag="f")
            nc.vector.tensor_scalar_add(u1[:], snc, EPS)
            nc.scalar.sqrt(u2[:], u1[:])
            nc.vector.tensor_scalar_add(u3[:], snc, 1.0)
            nc.vector.tensor_tensor(u1[:], u2[:], u3[:], MUL)
            nc.vector.reciprocal(u2[:], u1[:])
            nc.vector.tensor_tensor(f[:], snc, u2[:], MUL)
            return f

        def fill_v(va, vb, vc):
            """Copy padded v (rows 32k+o) into the block-diag stationaries.
            32-aligned partition bases everywhere; DVE reads PSUM fine."""
            for k in range(4):
                nc.vector.tensor_copy(
                    VAB0[32 * k : 32 * k + 16, 32 * k : 32 * k + 32],
                    va[32 * k : 32 * k + 16, :])
                nc.vector.tensor_copy(
                    VAB1[32 * k : 32 * k + 16, 32 * k : 32 * k + 32],
                    vb[32 * k : 32 * k + 16, :])
            for k in range(2):
                nc.vector.tensor_copy(
                    VC[32 * k : 32 * k + 16, 32 * k : 32 * k + 32],
                    vc[32 * k : 32 * k + 16, :])

        # ---------------------------------------------------------------
        def s_pass0():
            """it0: s[b, co] = sum_{r,i} x W (uniform routing folded later)."""
            for q in range(Q):
                for i in range(I):
                    st = (q == 0 and i == 0)
                    sp = (q == Q - 1 and i == I - 1)
                    nc.tensor.matmul(psC0[:], xtr[:, q, i, :],
                                     wfr8[:, q, i, :], start=st, stop=sp)
                    nc.tensor.matmul(psC1[:], xtr[:, q, i, :],
                                     wfr2[:, q, i, :], start=st, stop=sp)

        def squash0():
            """it0 squash in [b, (c,o)] layout; v transposed into padded form."""
            sC = smp.tile([BL, 160], F32, tag="sC")
            sqC = smp.tile([BL, 160], F32, tag="sqC")
            sn0 = smp.tile([BL, C], F32, tag="sn0")
            nc.scalar.copy(sC[:, 0:128], psC0)
            nc.scalar.copy(sC[:, 128:160], psC1)
            nc.scalar.square(sqC[:], sC[:])
            nc.vector.tensor_reduce(
                sn0[:], sqC.rearrange("b (c o) -> b c o", o=O),
                axis=mybir.AxisListType.X, op=ADD,
            )
            nc.vector.tensor_scalar_mul(sn0[:], sn0[:], 1.0 / (R * R))
            f = f_chain(sn0[:], BL, C)
            nc.vector.tensor_scalar_mul(f[:], f[:], 1.0 / R)
            nc.vector.tensor_tensor(
                v0p.rearrange("b (c oo) -> b c oo", oo=32)[:, :, 0:O],
                sC.rearrange("b (c o) -> b c o", o=O)[:, 0:8, :],
                f[:, 0:8].unsqueeze(-1).broadcast_to([BL, 8, O]),
                MUL,
            )
            nc.vector.tensor_tensor(
                v0p2.rearrange("b (c oo) -> b c oo", oo=32)[:, :, 0:O],
                sC.rearrange("b (c o) -> b c o", o=O)[:, 8:10, :],
                f[:, 8:10].unsqueeze(-1).broadcast_to([BL, 2, O]),
                MUL,
            )
            nc.tensor.matmul(psFa, v0p[:, 0:128], id16[0:BL, 0:BL],
                             start=True, stop=True)
            nc.tensor.matmul(psFb, v0p[:, 128:256], id16[0:BL, 0:BL],
                             start=True, stop=True)
            nc.tensor.matmul(psFc, v0p2[:], id16[0:BL, 0:BL],
                             start=True, stop=True)
            fill_v(psFa, psFb, psFc)

        # ---------------------------------------------------------------
        def build_y(q):
            """y8[rr,q,i,(c,b)] = cw[c,b,r]*x[b,r,i] classes 0-7; y2 for 8,9."""
            cw8 = (
                cwT[:, q, 0:2, :]
                .rearrange("p w (k b) -> p (w k) b", b=BL)
                .unsqueeze(1)
                .broadcast_to([128, I, 8, BL])
            )
            xv = xtr[:, q].unsqueeze(2)
            eng = nc.vector if q % 2 == 0 else nc.gpsimd
            eng.tensor_tensor(
                y8[:, q].rearrange("p i (c b) -> p i c b", b=BL),
                cw8,
                xv.broadcast_to([128, I, 8, BL]),
                MUL,
            )
            cw2 = (
                cwT[:, q, 2, 0:64]
                .rearrange("p (k b) -> p k b", b=BL)
                .unsqueeze(1)
                .broadcast_to([128, I, 2, BL])
            )
            nc.vector.tensor_tensor(
                y2[:, q].rearrange("p i (c b) -> p i c b", b=BL),
                cw2,
                xv.broadcast_to([128, I, 2, BL]),
                MUL,
            )

        def s_pass(it):
            """it>0: shared W stationaries; diag blocks of psA/psB are s."""
            for q in range(Q):
                build_y(q)
            for q in range(Q):
                for i in range(I):
                    st = (q == 0 and i == 0)
                    sp = (q == Q - 1 and i == I - 1)
                    nc.tensor.matmul(psA, wfr8[:, q, i, :], y8[:, q, i, :],
                                     start=st, stop=sp)
                    nc.tensor.matmul(psB, wfr2[:, q, i, :], y2[:, q, i, :],
                                     start=st, stop=sp)

        def squash_co(it):
            """it1/2 squash via PE row-permutation into padded layout."""
            sA = smp.tile([128, 256], F32, tag="sA")
            sB = smp.tile([32, 64], F32, tag="sB")
            nc.scalar.copy(sA[:], psA)
            nc.scalar.copy(sB[:], psB)
            # permute rows 16c+o -> 32k+o so diag blocks sit 32-aligned
            nc.tensor.matmul(psT6[:], p0[:], sA[:], start=True, stop=True)
            nc.tensor.matmul(psT7[:], p1[:], sA[:], start=True, stop=True)
            nc.tensor.matmul(psP2, p2[:], sB[:], start=True, stop=True)
            for k in range(4):
                nc.vector.tensor_copy(
                    s1p0[32 * k : 32 * k + 16, :],
                    psT6[32 * k : 32 * k + 16, 32 * k : 32 * k + 32])
                nc.vector.tensor_copy(
                    s1p1[32 * k : 32 * k + 16, :],
                    psT7[32 * k : 32 * k + 16, 128 + 32 * k : 128 + 32 * k + 32])
            for k in range(2):
                nc.vector.tensor_copy(
                    s2p[32 * k : 32 * k + 16, :],
                    psP2[32 * k : 32 * k + 16, 32 * k : 32 * k + 32])
            nc.scalar.square(sqcat[:, 0:32], s1p0[:])
            nc.scalar.square(sqcat[:, 32:64], s1p1[:])
            nc.scalar.square(sqcat[0:64, 64:96], s2p[:])
            nc.tensor.matmul(psn, e10[:], sqcat[:], start=True, stop=True)
            # f on all 96 cols at once; each class reads its own col-block
            f = f_chain(psn, C, 96)
            nc.tensor.matmul(psFa, efa[:], f[:, 0:32], start=True, stop=True)
            nc.tensor.matmul(psFb, efb[:], f[:, 32:64], start=True, stop=True)
            nc.tensor.matmul(psFc, efc[:], f[:, 64:96], start=True, stop=True)
            if it == 2:
                vpa = smp.tile([128, BL], F32, tag="vpa")
                vpb = smp.tile([128, BL], F32, tag="vpb")
                vpc = smp.tile([64, BL], F32, tag="vpc")
                nc.vector.tensor_tensor(vpa[:], s1p0[:], psFa, MUL)
                nc.vector.tensor_tensor(vpb[:], s1p1[:], psFb, MUL)
                nc.vector.tensor_tensor(vpc[:], s2p[:], psFc, MUL)
                for k in range(4):
                    nc.sync.dma_start(out_d[k],
                                      vpa[32 * k : 32 * k + 16, :])
                    nc.sync.dma_start(out_d[4 + k],
                                      vpb[32 * k : 32 * k + 16, :])
                for k in range(2):
                    nc.sync.dma_start(out_d[8 + k],
                                      vpc[32 * k : 32 * k + 16, :])
            else:
                va = smp.tile([128, BL], F16, tag="va")
                vb = smp.tile([128, BL], F16, tag="vb")
                vc = smp.tile([64, BL], F16, tag="vc")
                nc.vector.tensor_tensor(va[:], s1p0[:], psFa, MUL)
                nc.vector.tensor_tensor(vb[:], s1p1[:], psFb, MUL)
                nc.vector.tensor_tensor(vc[:], s2p[:], psFc, MUL)
                fill_v(va, vb, vc)

        # ---------------------------------------------------------------
        def agreement_softmax():
            """L[p,w,r] += sum_i x*(sum_o v*W); then per-wave softmax to cwT."""
            vsrc = (VAB0[:], VAB1[:], VC[:])
            msrc = (wtg0[:], wtg1[:], wtg2[:])
            cwvs = []
            for w in range(3):
                rows = W_ROWS[w]
                for n in range(NG):
                    off = n * GCH
                    pu = psU.tile([128, GCH], F32, tag="pu")
                    # fp16 moving operand caps at 512 cols; split the chunk
                    for h in range(0, GCH, 512):
                        nc.tensor.matmul(pu[0:rows, h : h + 512], vsrc[w],
                                         msrc[w][:, off + h : off + h + 512],
                                         start=True, stop=True)
                    gm = gmp.tile([128, GCH], F16, tag="gm")
                    nc.vector.tensor_tensor(
                        gm[0:rows, :], pu[0:rows, :],
                        xrep[0:rows, off : off + GCH], MUL,
                    )
                    l1 = trp.tile([128, GCH // 2], F16, tag="l1")
                    l2 = trp.tile([128, GCH // 4], F16, tag="l2")
                    a = trp.tile([128, GCH // 8], F32, tag="a")
                    gmv = gm.rearrange("p (r i) -> p r i", i=I)
                    l1v = l1.rearrange("p (r i) -> p r i", i=4)
                    l2v = l2.rearrange("p (r i) -> p r i", i=2)
                    nc.gpsimd.tensor_tensor(
                        l1v[0:rows], gmv[0:rows, :, 0:4], gmv[0:rows, :, 4:8], ADD
                    )
                    nc.gpsimd.tensor_tensor(
                        l2v[0:rows], l1v[0:rows, :, 0:2], l1v[0:rows, :, 2:4], ADD
                    )
                    nc.gpsimd.tensor_tensor(
                        a[0:rows], l2v[0:rows, :, 0], l2v[0:rows, :, 1], ADD
                    )
                    r0 = off // I
                    nr = GCH // I
                    nc.gpsimd.tensor_tensor(
                        L[0:rows, w, r0 : r0 + nr],
                        L[0:rows, w, r0 : r0 + nr],
                        a[0:rows],
                        ADD,
                    )
                # per-wave softmax pieces right after the wave's chunks
                cwv = cwp.tile([128, R], F32, tag="cwv")
                Zt = smp.tile([128, 1], F32, tag="Zt")
                Zi = smp.tile([128, 1], F32, tag="Zi")
                nc.scalar.activation(cwv[0:rows, :], L[0:rows, w, :], AF.Exp,
                                     accum_out=Zt[0:rows])
                nc.vector.reciprocal(Zi[0:rows], Zt[0:rows])
                nc.vector.tensor_scalar_mul(cwv[0:rows, :], cwv[0:rows, :],
                                            Zi[0:rows])
                cwvs.append((cwv, rows))
            # transposes last so PE isn't stalled between waves
            for w in range(3):
                cwv, rows = cwvs[w]
                for q in range(Q):
                    pt = (psT6, psT7)[q % 2]
                    nc.tensor.matmul(pt[:, 0:rows],
                                     cwv[0:rows, 128 * q : 128 * (q + 1)],
                                     id32[0:rows, 0:rows], start=True, stop=True)
                    nc.scalar.copy(cwT[:, q, w, 0:rows], pt[:, 0:rows])

        # =========================== flow ==============================
        s_pass0()
        squash0()
        agreement_softmax()
        s_pass(1)
        squash_co(it=1)
        agreement_softmax()
        s_pass(2)
        squash_co(it=2)

    nc.compile()
    return nc


# =================== host-side prep / entry point =====================

def _prep_shared(W):
    """Per-problem constant tensors (replicated on every core)."""
    W = np.ascontiguousarray(W, np.float32)
    wfr8 = np.ascontiguousarray(
        W[:8].reshape(8, Q, 128, I, O).transpose(2, 1, 3, 0, 4).reshape(128, Q, I, 128)
    ).astype(np.float16)
    wfr2 = np.ascontiguousarray(
        W[8:].reshape(2, Q, 128, I, O).transpose(2, 1, 3, 0, 4).reshape(128, Q, I, 32)
    ).astype(np.float16)
    # padded 32-row class slots
    wtg0 = np.zeros((128, RI), np.float16)
    wtg1 = np.zeros((128, RI), np.float16)
    wtg2 = np.zeros((64, RI), np.float16)
    for k in range(4):
        wtg0[32 * k : 32 * k + 16] = W[k].transpose(2, 0, 1).reshape(O, RI)
        wtg1[32 * k : 32 * k + 16] = W[4 + k].transpose(2, 0, 1).reshape(O, RI)
    for k in range(2):
        wtg2[32 * k : 32 * k + 16] = W[8 + k].transpose(2, 0, 1).reshape(O, RI)
    id16 = np.eye(128, dtype=np.float16)
    id32 = np.eye(128, dtype=np.float32)
    # row permutations compact [16c+o] -> padded [32k+o]
    p0 = np.zeros((128, 128), np.float32)
    p1 = np.zeros((128, 128), np.float32)
    p2 = np.zeros((32, 64), np.float32)
    for o in range(O):
        for k in range(4):
            p0[16 * k + o, 32 * k + o] = 1.0
            p1[16 * (4 + k) + o, 32 * k + o] = 1.0
        for k in range(2):
            p2[16 * k + o, 32 * k + o] = 1.0
    # per-class norm reduce: psn[c, :] = sum_o sq[32k+o, :]
    e10 = np.zeros((128, C), np.float32)
    for o in range(O):
        for k in range(4):
            e10[32 * k + o, k] = 1.0
            e10[32 * k + o, 4 + k] = 1.0
        for k in range(2):
            e10[32 * k + o, 8 + k] = 1.0
    # padded frep: frep[32k+oo] = f[class(k)] for all oo
    efa = np.zeros((C, 128), np.float32)
    efb = np.zeros((C, 128), np.float32)
    efc = np.zeros((C, 64), np.float32)
    for k in range(4):
        efa[k, 32 * k : 32 * k + 32] = 1.0
        efb[4 + k, 32 * k : 32 * k + 32] = 1.0
    for k in range(2):
        efc[8 + k, 32 * k : 32 * k + 32] = 1.0
    return {
        "wfr8": wfr8, "wfr2": wfr2, "wtg0": wtg0, "wtg1": wtg1, "wtg2": wtg2,
        "id16": id16, "id32": id32, "p0": p0, "p1": p1, "p2": p2,
        "e10": e10, "efa": efa, "efb": efb, "efc": efc,
    }


def _prep_core(x_shard):
    """Per-core tensors for one 32-batch shard."""
    xs = np.ascontiguousarray(x_shard, np.float32)       # [32, 1152, 8]
    xtr = np.ascontiguousarray(
        xs.reshape(BL, Q, 128, I).transpose(2, 1, 3, 0)
    ).astype(np.float16)                                  # [128, Q, I, 32]
    flat = xs.reshape(BL, RI)
    xrep = np.ascontiguousarray(
        flat[np.arange(128) % BL].astype(np.float16)
    )                                                     # [128, RI]
    return {"xtr": xtr, "xrep": xrep}


_NC_CACHE = {}


def kernel(x, W):
    x = np.asarray(x, np.float32)
    W = np.asarray(W, np.float32)
    if "nc" not in _NC_CACHE:
        _NC_CACHE["nc"] = build_nc()
    nc = _NC_CACHE["nc"]

    shared = _prep_shared(W)
    in_maps = []
    for m in range(NC):
        per = _prep_core(x[m * BL : (m + 1) * BL])
        in_maps.append({**shared, **per})

    res = run_bass_kernel_spmd(nc, in_maps, list(range(NC)))
    out = np.empty((C, B, 1, 1, O), np.float32)
    for m in range(NC):
        o = res.results[m]["out"]                         # [C, O, BL]
        out[:, m * BL : (m + 1) * BL, 0, 0, :] = np.asarray(o).transpose(0, 2, 1)
    return out


if __name__ == "__main__":
    d = np.load("/root/problem/ref_data.npz")
    got = kernel(d["x"], d["W"])
    exp = d["expected"]
    err = np.abs(got - exp).max() / np.abs(exp).max()
    print("Relative error:", err)


# revision 19
# speedup vs baseline: 1.2991x; 1.0928x over previous
"""Trainium2 Bass kernel for nn_CapsuleLayer (dynamic routing), v3.

Problem:  u_hat = einsum('bri,crio->cbro', x, W);  3 routing iterations
          (softmax over R, weighted sum, squash, agreement update).
Shapes:   x [256, 1152, 8] f32, W [10, 1152, 8, 16] f32 ->
          out [10, 256, 1, 1, 16] f32.

Strategy (8 NeuronCores, data-parallel over batch, B_loc = 32/core):
  * all matmul operands fp16 (PE 16-bit streams 1 col/cycle vs fp32 4x);
    accumulation fp32 in PSUM, logits L accumulate fp32 in SBUF;
  * s-pass it0: x-chunk stationaries [128,32] against W moving [128,160]
    giving s[b, co] directly; it1/2: shared W stationaries [128,128]
    covering classes 0-7 with a [128,256] moving of per-class weighted x
    (y8); the (class,class) diagonal blocks of the PSUM result are s;
  * every engine op needs a 32-aligned partition base, so per-class
    16-row data lives in padded 32-row slots (top 16 = data, bottom 16 =
    zeros); the s diagonal is rearranged into that form with PE
    permutation matmuls, per-class norms go through one concatenated
    [128,96] square tile and a single [10,96]-output matmul;
  * agreement: per-wave padded block-diagonal v stationary (3 LDWEIGHTS
    per agreement) streaming resident padded wt in 1024-col chunks;
    U*x on DVE, i-reduction tree on GpSimd in fp16, L accumulated fp32;
  * softmax: ACT exp (fp32, accumulated Z); 1/Z is folded into the PE
    transpose by using diag(1/Z) as the matmul rhs; cwT stored fp16.
"""

import sys
from contextlib import ExitStack

import numpy as np

sys.path.insert(0, "/opt/trn_rl_repo")

import concourse.bacc as bacc
import concourse.bass as bass
import concourse.mybir as mybir
import concourse.tile as tile
from concourse.bass_utils import run_bass_kernel_spmd

F32 = mybir.dt.float32
F16 = mybir.dt.float16
MUL = mybir.AluOpType.mult
ADD = mybir.AluOpType.add
AF = mybir.ActivationFunctionType

B, R, I, C, O = 256, 1152, 8, 10, 16
NC = 8
BL = B // NC          # 32 batch per core
Q = R // 128          # 9 r-blocks of 128
RI = R * I            # 9216
GCH = 1024            # agreement (r,i) chunk
NG = RI // GCH        # 9 chunks
EPS = 1e-7
W_ROWS = (128, 128, 64)   # U-matmul output rows per wave ((k,b) pairs)


def build_nc(debug=False):
    nc = bacc.Bacc("TRN2", target_bir_lowering=False, debug=debug)

    def din(name, shape, dt=F16):
        return nc.declare_dram_parameter(name, shape, dt, isOutput=False)

    xtr_d = din("xtr", [128, Q, I, BL])
    wfr8_d = din("wfr8", [128, Q, I, 128])
    wfr2_d = din("wfr2", [128, Q, I, 32])
    xrep_d = din("xrep", [128, RI])
    wtg0_d = din("wtg0", [128, RI])
    wtg1_d = din("wtg1", [128, RI])
    wtg2_d = din("wtg2", [64, RI])
    id16_d = din("id16", [128, 128])
    id32_d = din("id32", [128, 128], F32)
    p0_d = din("p0", [128, 128], F32)
    p1_d = din("p1", [128, 128], F32)
    p2_d = din("p2", [32, 64], F32)
    e10_d = din("e10", [128, C], F32)
    efa_d = din("efa", [C, 128], F32)
    efb_d = din("efb", [C, 128], F32)
    efc_d = din("efc", [C, 64], F32)
    out_d = nc.declare_dram_parameter("out", [C, O, BL], F32, isOutput=True)

    with tile.TileContext(nc) as tc, ExitStack() as ctx:
        res = ctx.enter_context(tc.tile_pool(name="res", bufs=1))
        cwp = ctx.enter_context(tc.tile_pool(name="cwp", bufs=3))
        y8p = ctx.enter_context(tc.tile_pool(name="y8p", bufs=2))
        ump = ctx.enter_context(tc.tile_pool(name="ump", bufs=2))
        trp = ctx.enter_context(tc.tile_pool(name="trp", bufs=1))
        smp = ctx.enter_context(tc.tile_pool(name="smp", bufs=1))
        psU = ctx.enter_context(
            tc.tile_pool(name="psU", bufs=2, space=bass.MemorySpace.PSUM)
        )
        psP = ctx.enter_context(
            tc.tile_pool(name="psP", bufs=1, space=bass.MemorySpace.PSUM)
        )

        # ---- resident tensors -------------------------------------
        xtr = res.tile([128, Q, I, BL], F16)
        wfr8 = res.tile([128, Q, I, 128], F16)
        wfr2 = res.tile([128, Q, I, 32], F16)
        xrep = res.tile([128, RI], F16)
        wtg0 = res.tile([128, RI], F16)
        wtg1 = res.tile([128, RI], F16)
        wtg2 = res.tile([64, RI], F16)
        id16 = res.tile([128, 128], F16)
        id32 = res.tile([128, 128], F32)
        p0 = res.tile([128, 128], F32)
        p1 = res.tile([128, 128], F32)
        p2 = res.tile([32, 64], F32)
        e10 = res.tile([128, C], F32)
        efa = res.tile([C, 128], F32)
        efb = res.tile([C, 128], F32)
        efc = res.tile([C, 64], F32)
        L = res.tile([128, 3, R], F32)
        cwT = res.tile([128, Q, 3, 128], F16)
        gm2 = res.tile([128, I, R], F16)  # per-wave U*x, (i, r) order
        # padded per-class storage: rows 32k..32k+16 = data, rest zero
        VAB0 = res.tile([128, 128], F16)  # v stationary, classes 0-3
        VAB1 = res.tile([128, 128], F16)  # classes 4-7
        VC = res.tile([64, 64], F16)      # classes 8-9
        s1p0 = res.tile([128, BL], F32)   # padded s, classes 0-3
        s1p1 = res.tile([128, BL], F32)   # classes 4-7
        s2p = res.tile([64, BL], F32)     # classes 8-9
        sqcat = res.tile([128, 96], F32)  # squares, 3 col-blocks
        v0p = res.tile([BL, 256], F16)    # it0 v, col-padded (c, 32)
        v0p2 = res.tile([BL, 64], F16)

        # PSUM (8 banks x 2KB). psU: 2 bufs x [128,1024]f32 = banks 0-3.
        # ps1 bank0 (cols 0:512): psA s-accumulator. bank1 (cols 512:1024):
        # psB/psC accumulators + single-shot outputs. A group's start=True
        # clears has_written bank-wide, so concurrently-accumulating groups
        # (psA vs psB) sit in different banks; single-shot outputs only
        # lose has_written bits, never data.
        ps1 = psP.tile([128, 1024], F32)
        psA = ps1[:, 0:256]
        psB = ps1[0:32, 512:576]
        # psC's two concurrently-accumulating groups must sit in different
        # banks (start=True clears has_written bank-wide)
        psC0 = ps1[0:BL, 256:384]        # bank 0 (idle during it0)
        psC1 = ps1[0:BL, 576:608]        # bank 1
        psP2 = ps1[0:64, 736:800]        # permuted sB
        psn = ps1[0:C, 800:896]          # [10, 96] norms
        psFa = ps1[:, 896:928]           # padded frep / it0 transposed v
        psFb = ps1[:, 928:960]
        psFc = ps1[0:64, 960:992]
        # transpose slots in separate banks: PE writing a bank while another
        # engine reads the same bank is fatal. Also hold permuted sA halves.
        psT6 = psP.tile([128, 256], F32)
        psT7 = psP.tile([128, 256], F32)

        # input DMAs: per-q for the s-pass tensors so compute starts early
        for q in range(Q):
            nc.sync.dma_start(xtr[:, q], xtr_d[:, q])
            nc.sync.dma_start(wfr8[:, q], wfr8_d[:, q])
            nc.sync.dma_start(wfr2[:, q], wfr2_d[:, q])
        for t, d in ((id16, id16_d), (id32, id32_d), (p0, p0_d), (p1, p1_d),
                     (p2, p2_d), (e10, e10_d), (efa, efa_d), (efb, efb_d),
                     (efc, efc_d), (wtg0, wtg0_d), (xrep, xrep_d),
                     (wtg1, wtg1_d), (wtg2, wtg2_d)):
            nc.sync.dma_start(t[:], d[:])
        nc.vector.memset(L[:], 0.0)
        nc.gpsimd.memset(VAB0[:], 0.0)
        nc.gpsimd.memset(VAB1[:], 0.0)
        nc.gpsimd.memset(VC[:], 0.0)
        nc.gpsimd.memset(s1p0[:], 0.0)
        nc.gpsimd.memset(s1p1[:], 0.0)
        nc.gpsimd.memset(s2p[:], 0.0)
        nc.gpsimd.memset(sqcat[:], 0.0)
        nc.gpsimd.memset(v0p[:], 0.0)
        nc.gpsimd.memset(v0p2[:], 0.0)

        # ---------------------------------------------------------------
        def f_chain(snc, p, n):
            """f = (sn/(1+sn)) / sqrt(sn+eps) elementwise on [p, n]."""
            u1 = smp.tile([p, n], F32, tag="u1")
            u2 = smp.tile([p, n], F32, tag="u2")
            u3 = smp.tile([p, n], F32, tag="u3")
            f = smp.tile([p, n], F32, tag="f")
            nc.vector.tensor_scalar_add(u1[:], snc, EPS)
            nc.scalar.sqrt(u2[:], u1[:])
            nc.vector.tensor_scalar_add(u3[:], snc, 1.0)
            nc.vector.tensor_tensor(u1[:], u2[:], u3[:], MUL)
            nc.vector.reciprocal(u2[:], u1[:])
            nc.vector.tensor_tensor(f[:], snc, u2[:], MUL)
            return f

        def fill_v(va, vb, vc):
            """Copy padded v (rows 32k+o) into the block-diag stationaries.
            32-aligned partition bases everywhere; DVE reads PSUM fine."""
            for k in range(4):
                nc.vector.tensor_copy(
                    VAB0[32 * k : 32 * k + 16, 32 * k : 32 * k + 32],
                    va[32 * k : 32 * k + 16, :])
                nc.vector.tensor_copy(
                    VAB1[32 * k : 32 * k + 16, 32 * k : 32 * k + 32],
                    vb[32 * k : 32 * k + 16, :])
            for k in range(2):
                nc.vector.tensor_copy(
                    VC[32 * k : 32 * k + 16, 32 * k : 32 * k + 32],
                    vc[32 * k : 32 * k + 16, :])

        # ---------------------------------------------------------------
        def s_pass0():
            """it0: s[b, co] = sum_{r,i} x W (uniform routing folded later)."""
            for q in range(Q):
                for i in range(I):
                    st = (q == 0 and i == 0)
                    sp = (q == Q - 1 and i == I - 1)
                    nc.tensor.matmul(psC0[:], xtr[:, q, i, :],
                                     wfr8[:, q, i, :], start=st, stop=sp)
                    nc.tensor.matmul(psC1[:], xtr[:, q, i, :],
                                     wfr2[:, q, i, :], start=st, stop=sp)

        def squash0():
            """it0 squash in [b, (c,o)] layout; v transposed into padded form."""
            sC = smp.tile([BL, 160], F32, tag="sC")
            sqC = smp.tile([BL, 160], F32, tag="sqC")
            sn0 = smp.tile([BL, C], F32, tag="sn0")
            nc.scalar.copy(sC[:, 0:128], psC0)
            nc.scalar.copy(sC[:, 128:160], psC1)
            nc.scalar.square(sqC[:], sC[:])
            nc.vector.tensor_reduce(
                sn0[:], sqC.rearrange("b (c o) -> b c o", o=O),
                axis=mybir.AxisListType.X, op=ADD,
            )
            nc.vector.tensor_scalar_mul(sn0[:], sn0[:], 1.0 / (R * R))
            f = f_chain(sn0[:], BL, C)
            nc.vector.tensor_scalar_mul(f[:], f[:], 1.0 / R)
            nc.vector.tensor_tensor(
                v0p.rearrange("b (c oo) -> b c oo", oo=32)[:, :, 0:O],
                sC.rearrange("b (c o) -> b c o", o=O)[:, 0:8, :],
                f[:, 0:8].unsqueeze(-1).broadcast_to([BL, 8, O]),
                MUL,
            )
            nc.vector.tensor_tensor(
                v0p2.rearrange("b (c oo) -> b c oo", oo=32)[:, :, 0:O],
                sC.rearrange("b (c o) -> b c o", o=O)[:, 8:10, :],
                f[:, 8:10].unsqueeze(-1).broadcast_to([BL, 2, O]),
                MUL,
            )
            nc.tensor.matmul(psFa, v0p[:, 0:128], id16[0:BL, 0:BL],
                             start=True, stop=True)
            nc.tensor.matmul(psFb, v0p[:, 128:256], id16[0:BL, 0:BL],
                             start=True, stop=True)
            nc.tensor.matmul(psFc, v0p2[:], id16[0:BL, 0:BL],
                             start=True, stop=True)
            fill_v(psFa, psFb, psFc)

        # ---------------------------------------------------------------
        def build_y(q):
            """y8[i,(c,b)] = cw[c,b,r]*x[b,r,i] classes 0-7; y2 for 8,9."""
            y8 = y8p.tile([128, I, 256], F16, tag="y8")
            y2 = y8p.tile([128, I, 64], F16, tag="y2")
            cw8 = (
                cwT[:, q, 0:2, :]
                .rearrange("p w (k b) -> p (w k) b", b=BL)
                .unsqueeze(1)
                .broadcast_to([128, I, 8, BL])
            )
            xv = xtr[:, q].unsqueeze(2)
            eng = nc.vector if q % 2 == 0 else nc.gpsimd
            eng.tensor_tensor(
                y8.rearrange("p i (c b) -> p i c b", b=BL),
                cw8,
                xv.broadcast_to([128, I, 8, BL]),
                MUL,
            )
            cw2 = (
                cwT[:, q, 2, 0:64]
                .rearrange("p (k b) -> p k b", b=BL)
                .unsqueeze(1)
                .broadcast_to([128, I, 2, BL])
            )
            nc.vector.tensor_tensor(
                y2.rearrange("p i (c b) -> p i c b", b=BL),
                cw2,
                xv.broadcast_to([128, I, 2, BL]),
                MUL,
            )
            return y8, y2

        def s_pass(it):
            """it>0: shared W stationaries; diag blocks of psA/psB are s."""
            for q in range(Q):
                y8, y2 = build_y(q)
                for i in range(I):
                    st = (q == 0 and i == 0)
                    sp = (q == Q - 1 and i == I - 1)
                    nc.tensor.matmul(psA, wfr8[:, q, i, :], y8[:, i, :],
                                     start=st, stop=sp)
                    nc.tensor.matmul(psB, wfr2[:, q, i, :], y2[:, i, :],
                                     start=st, stop=sp)

        def squash_co(it):
            """it1/2 squash via PE row-permutation into padded layout."""
            sA = smp.tile([128, 256], F32, tag="sA")
            sB = smp.tile([32, 64], F32, tag="sB")
            nc.scalar.copy(sA[:], psA)
            nc.scalar.copy(sB[:], psB)
            # permute rows 16c+o -> 32k+o so diag blocks sit 32-aligned
            nc.tensor.matmul(psT6[:], p0[:], sA[:], start=True, stop=True)
            nc.tensor.matmul(psT7[:], p1[:], sA[:], start=True, stop=True)
            nc.tensor.matmul(psP2, p2[:], sB[:], start=True, stop=True)
            for k in range(4):
                nc.vector.tensor_copy(
                    s1p0[32 * k : 32 * k + 16, :],
                    psT6[32 * k : 32 * k + 16, 32 * k : 32 * k + 32])
                nc.vector.tensor_copy(
                    s1p1[32 * k : 32 * k + 16, :],
                    psT7[32 * k : 32 * k + 16, 128 + 32 * k : 128 + 32 * k + 32])
            for k in range(2):
                nc.vector.tensor_copy(
                    s2p[32 * k : 32 * k + 16, :],
                    psP2[32 * k : 32 * k + 16, 32 * k : 32 * k + 32])
            nc.scalar.square(sqcat[:, 0:32], s1p0[:])
            nc.scalar.square(sqcat[:, 32:64], s1p1[:])
            nc.scalar.square(sqcat[0:64, 64:96], s2p[:])
            nc.tensor.matmul(psn, e10[:], sqcat[:], start=True, stop=True)
            # f on all 96 cols at once; each class reads its own col-block
            f = f_chain(psn, C, 96)
            nc.tensor.matmul(psFa, efa[:], f[:, 0:32], start=True, stop=True)
            nc.tensor.matmul(psFb, efb[:], f[:, 32:64], start=True, stop=True)
            nc.tensor.matmul(psFc, efc[:], f[:, 64:96], start=True, stop=True)
            if it == 2:
                vpa = smp.tile([128, BL], F32, tag="vpa")
                vpb = smp.tile([128, BL], F32, tag="vpb")
                vpc = smp.tile([64, BL], F32, tag="vpc")
                nc.vector.tensor_tensor(vpa[:], s1p0[:], psFa, MUL)
                nc.vector.tensor_tensor(vpb[:], s1p1[:], psFb, MUL)
                nc.vector.tensor_tensor(vpc[:], s2p[:], psFc, MUL)
                for k in range(4):
                    nc.sync.dma_start(out_d[k],
                                      vpa[32 * k : 32 * k + 16, :])
                    nc.sync.dma_start(out_d[4 + k],
                                      vpb[32 * k : 32 * k + 16, :])
                for k in range(2):
                    nc.sync.dma_start(out_d[8 + k],
                                      vpc[32 * k : 32 * k + 16, :])
            else:
                va = smp.tile([128, BL], F16, tag="va")
                vb = smp.tile([128, BL], F16, tag="vb")
                vc = smp.tile([64, BL], F16, tag="vc")
                nc.vector.tensor_tensor(va[:], s1p0[:], psFa, MUL)
                nc.vector.tensor_tensor(vb[:], s1p1[:], psFb, MUL)
                nc.vector.tensor_tensor(vc[:], s2p[:], psFc, MUL)
                fill_v(va, vb, vc)

        # ---------------------------------------------------------------
        def agreement_softmax():
            """L[p,w,r] += sum_i x*(sum_o v*W); then per-wave softmax to cwT."""
            vsrc = (VAB0[:], VAB1[:], VC[:])
            msrc = (wtg0[:], wtg1[:], wtg2[:])
            cwvs = []
            for w in range(3):
                rows = W_ROWS[w]
                gmf = gm2.rearrange("p i r -> p (i r)")
                for n in range(NG):
                    off = n * GCH
                    pu = psU.tile([128, GCH], F32, tag="pu")
                    # fp16 moving operand caps at 512 cols; split the chunk
                    for h in range(0, GCH, 512):
                        nc.tensor.matmul(pu[0:rows, h : h + 512], vsrc[w],
                                         msrc[w][:, off + h : off + h + 512],
                                         start=True, stop=True)
                    if n % 2 == 0:
                        # DVE multiplies straight out of PSUM
                        nc.vector.tensor_tensor(
                            gmf[0:rows, off : off + GCH], pu[0:rows, :],
                            xrep[0:rows, off : off + GCH], MUL,
                        )
                    else:
                        # ACT drains to fp16, GpSimd multiplies in SBUF
                        um = ump.tile([128, GCH], F16, tag="um")
                        nc.scalar.copy(um[0:rows, :], pu[0:rows, :])
                        nc.gpsimd.tensor_tensor(
                            gmf[0:rows, off : off + GCH], um[0:rows, :],
                            xrep[0:rows, off : off + GCH], MUL,
                        )
                # full-wave i-reduction: 3 contiguous fp16 adds + fp32 accum
                l1 = trp.tile([128, 4, R], F16, tag="l1")
                l2 = trp.tile([128, 2, R], F16, tag="l2")
                a = trp.tile([128, R], F16, tag="a")
                nc.gpsimd.tensor_tensor(
                    l1[0:rows], gm2[0:rows, 0:4, :], gm2[0:rows, 4:8, :], ADD
                )
                nc.vector.tensor_tensor(
                    l2[0:rows], l1[0:rows, 0:2, :], l1[0:rows, 2:4, :], ADD
                )
                nc.gpsimd.tensor_tensor(
                    a[0:rows], l2[0:rows, 0, :], l2[0:rows, 1, :], ADD
                )
                nc.vector.tensor_tensor(
                    L[0:rows, w, :], L[0:rows, w, :], a[0:rows], ADD
                )
                # per-wave softmax pieces right after the wave's chunks
                cwv = cwp.tile([128, R], F32, tag="cwv")
                Zt = smp.tile([128, 1], F32, tag="Zt")
                Zi = smp.tile([128, 1], F32, tag="Zi")
                nc.scalar.activation(cwv[0:rows, :], L[0:rows, w, :], AF.Exp,
                                     accum_out=Zt[0:rows])
                nc.vector.reciprocal(Zi[0:rows], Zt[0:rows])
                nc.vector.tensor_scalar_mul(cwv[0:rows, :], cwv[0:rows, :],
                                            Zi[0:rows])
                cwvs.append((cwv, rows))
            # transposes last so PE isn't stalled between waves
            for w in range(3):
                cwv, rows = cwvs[w]
                for q in range(Q):
                    pt = (psT6, psT7)[q % 2]
                    nc.tensor.matmul(pt[:, 0:rows],
                                     cwv[0:rows, 128 * q : 128 * (q + 1)],
                                     id32[0:rows, 0:rows], start=True, stop=True)
                    nc.scalar.copy(cwT[:, q, w, 0:rows], pt[:, 0:rows])

        # =========================== flow ==============================
        s_pass0()
        squash0()
        agreement_softmax()
        s_pass(1)
        squash_co(it=1)
        agreement_softmax()
        s_pass(2)
        squash_co(it=2)

    nc.compile()
    return nc


# =================== host-side prep / entry point =====================

def _prep_shared(W):
    """Per-problem constant tensors (replicated on every core)."""
    W = np.ascontiguousarray(W, np.float32)
    wfr8 = np.ascontiguousarray(
        W[:8].reshape(8, Q, 128, I, O).transpose(2, 1, 3, 0, 4).reshape(128, Q, I, 128)
    ).astype(np.float16)
    wfr2 = np.ascontiguousarray(
        W[8:].reshape(2, Q, 128, I, O).transpose(2, 1, 3, 0, 4).reshape(128, Q, I, 32)
    ).astype(np.float16)
    # padded 32-row class slots
    wtg0 = np.zeros((128, RI), np.float16)
    wtg1 = np.zeros((128, RI), np.float16)
    wtg2 = np.zeros((64, RI), np.float16)
    # column order (i, r): col = i*R + r  (makes the i-reduce contiguous)
    for k in range(4):
        wtg0[32 * k : 32 * k + 16] = W[k].transpose(2, 1, 0).reshape(O, RI)
        wtg1[32 * k : 32 * k + 16] = W[4 + k].transpose(2, 1, 0).reshape(O, RI)
    for k in range(2):
        wtg2[32 * k : 32 * k + 16] = W[8 + k].transpose(2, 1, 0).reshape(O, RI)
    id16 = np.eye(128, dtype=np.float16)
    id32 = np.eye(128, dtype=np.float32)
    # row permutations compact [16c+o] -> padded [32k+o]
    p0 = np.zeros((128, 128), np.float32)
    p1 = np.zeros((128, 128), np.float32)
    p2 = np.zeros((32, 64), np.float32)
    for o in range(O):
        for k in range(4):
            p0[16 * k + o, 32 * k + o] = 1.0
            p1[16 * (4 + k) + o, 32 * k + o] = 1.0
        for k in range(2):
            p2[16 * k + o, 32 * k + o] = 1.0
    # per-class norm reduce: psn[c, :] = sum_o sq[32k+o, :]
    e10 = np.zeros((128, C), np.float32)
    for o in range(O):
        for k in range(4):
            e10[32 * k + o, k] = 1.0
            e10[32 * k + o, 4 + k] = 1.0
        for k in range(2):
            e10[32 * k + o, 8 + k] = 1.0
    # padded frep: frep[32k+oo] = f[class(k)] for all oo
    efa = np.zeros((C, 128), np.float32)
    efb = np.zeros((C, 128), np.float32)
    efc = np.zeros((C, 64), np.float32)
    for k in range(4):
        efa[k, 32 * k : 32 * k + 32] = 1.0
        efb[4 + k, 32 * k : 32 * k + 32] = 1.0
    for k in range(2):
        efc[8 + k, 32 * k : 32 * k + 32] = 1.0
    return {
        "wfr8": wfr8, "wfr2": wfr2, "wtg0": wtg0, "wtg1": wtg1, "wtg2": wtg2,
        "id16": id16, "id32": id32, "p0": p0, "p1": p1, "p2": p2,
        "e10": e10, "efa": efa, "efb": efb, "efc": efc,
    }


def _prep_core(x_shard):
    """Per-core tensors for one 32-batch shard."""
    xs = np.ascontiguousarray(x_shard, np.float32)       # [32, 1152, 8]
    xtr = np.ascontiguousarray(
        xs.reshape(BL, Q, 128, I).transpose(2, 1, 3, 0)
    ).astype(np.float16)                                  # [128, Q, I, 32]
    flat = xs.transpose(0, 2, 1).reshape(BL, RI)          # (i, r) order
    xrep = np.ascontiguousarray(
        flat[np.arange(128) % BL].astype(np.float16)
    )                                                     # [128, RI]
    return {"xtr": xtr, "xrep": xrep}


_NC_CACHE = {}


def kernel(x, W):
    x = np.asarray(x, np.float32)
    W = np.asarray(W, np.float32)
    if "nc" not in _NC_CACHE:
        _NC_CACHE["nc"] = build_nc()
    nc = _NC_CACHE["nc"]

    shared = _prep_shared(W)
    in_maps = []
    for m in range(NC):
        per = _prep_core(x[m * BL : (m + 1) * BL])
        in_maps.append({**shared, **per})

    res = run_bass_kernel_spmd(nc, in_maps, list(range(NC)))
    out = np.empty((C, B, 1, 1, O), np.float32)
    for m in range(NC):
        o = res.results[m]["out"]                         # [C, O, BL]
        out[:, m * BL : (m + 1) * BL, 0, 0, :] = np.asarray(o).transpose(0, 2, 1)
    return out


if __name__ == "__main__":
    d = np.load("/root/problem/ref_data.npz")
    got = kernel(d["x"], d["W"])
    exp = d["expected"]
    err = np.abs(got - exp).max() / np.abs(exp).max()
    print("Relative error:", err)
